# revision 13
# baseline (speedup 1.0000x reference)
"""GQA int8-KV-cache decode attention on 8 NeuronCores (Bass/Tile), v3.

Sharding: kv-head axis (8 kv heads -> 1 per core), tensor parallel.
Host does tiny prep (RoPE + int8 quant of the new token, cache patch,
layout shuffles); device does the masked attention sweep.

Layout/algorithm choices:
  * K/V caches shipped to HBM as bf16 (cast on host). The DMA engines
    charge the max(src,dst) side, so an int8->bf16 cast DMA costs the
    same as a raw bf16 copy -- but raw copies ride the hardware DGE
    (no ~1us/DMA SWDGE desc-gen on GpSimd, no queue DRAIN stalls), so
    exact per-batch loads with no group padding become free.
  * K stored pre-transposed [B, D, CACHE] so each chunk arrives as
    KT [d, s] and feeds matmul directly -- no PE transpose.
  * V stored int8 position-interleaved [B, 8, 128, 4, D] so SBUF
    partition p holds positions {g*512 + j*128 + p}: per-partition
    contiguous runs are 512B (full DMA bandwidth) while the SBUF tile
    keeps partition = position-within-chunk.
  * Two independent DMA streams keep the 16 DMA engines gapless: K
    (bf16) free-runs on the sync HWDGE queue while V (int8, cast to
    bf16 in-flight) free-runs on the GpSimd SWDGE queue. The scalar
    queue stays clean for the exp ACTIVATEs (a DMA issue blocks under
    DMA-engine backpressure, and any DMA queued behind ACTIVATEs gets
    throttled to compute pace -- both measured). Every batch gets its
    own resident SBUF tile (the full working set fits), so no
    buffer-reuse WAR can delay a DMA issue.
  * Scores for G=8 chunks accumulate into one PSUM tile; the k_scaler
    multiply is a single DVE tensor_tensor against a stride-0-broadcast
    [128, G, 1->4] AP, then ONE batched ACT exp per G chunks (the ACT
    per-instruction PSUM-access overhead was the v2 bottleneck).
  * Mask via scalers only: masked positions have k_scaler = 0 (so
    exp(0) = 1 exactly) and v_scaler = 0 (no numerator contribution);
    the denominator over-count is the masked-position count, which the
    host subtracts exactly. No mask tensor, no -inf bias.
  * Denominator = sum_s exp accumulated as a per-partition vector on
    DVE (reduce over chunks + add); host does the final 128-partition
    sum. No per-chunk PE matmul for the denominator.
  * Numerator ov[d, r] += V_chunk.T @ pexp accumulates in PSUM across
    chunks; numerator and denominator ship out unnormalized, host
    divides.
  * scores->exp->PV chain software-pipelined at group granularity so
    the PE never head-of-line blocks on ACT/DVE.
"""

import os

os.environ.setdefault("JAX_PLATFORMS", "cpu")

import math
import numpy as np

B, H, KVH, D, CACHE = 16, 32, 8, 128, 4096
NREP = H // KVH
NCORES = 8
CHUNK = 128
VG = 4  # V chunks interleaved per 512B-run group (int8)
NBG = 4  # batches per DMA group
G = int(os.environ.get("KERNEL_G", "8"))  # chunks per ACT/exp batch

_BUILD_CACHE = {}
LAST_RESULTS = None


def _rope(x, cos, sin):
    # x: [B, 1, Hx, D]; cos/sin: [B, 1, D//2]
    c = cos[:, :, None, :]
    s = sin[:, :, None, :]
    xe, xo = x[..., ::2], x[..., 1::2]
    re = xe * c - xo * s
    im = xe * s + xo * c
    return np.stack([re, im], axis=-1).reshape(x.shape).astype(np.float32)


def _build_program(ncs):
    """ncs: per-device-batch chunk counts, sorted ascending (same for
    every core since sharding is by kv head)."""
    from contextlib import ExitStack

    import concourse.bacc as bacc
    import concourse.tile as tile
    from concourse import mybir
    from concourse.bass import AP

    nc = bacc.Bacc()
    f32 = mybir.dt.float32
    bf16 = mybir.dt.bfloat16
    mult = mybir.AluOpType.mult
    add = mybir.AluOpType.add

    i8 = mybir.dt.int8
    kt16 = nc.dram_tensor("kt16", [B, D, CACHE], bf16, kind="ExternalInput")
    v8 = nc.dram_tensor(
        "v8", [B, CACHE // (VG * CHUNK), CHUNK, VG, D], i8, kind="ExternalInput"
    )
    qt = nc.dram_tensor("qt", [CHUNK, B, NREP], bf16, kind="ExternalInput")
    ksc = nc.dram_tensor("ksc", [CHUNK, B, CACHE // CHUNK], f32, kind="ExternalInput")
    vsc = nc.dram_tensor("vsc", [CHUNK, B, CACHE // CHUNK], f32, kind="ExternalInput")
    o_num = nc.dram_tensor("o_num", [CHUNK, B, NREP], f32, kind="ExternalOutput")
    o_den = nc.dram_tensor("o_den", [CHUNK, B, NREP], f32, kind="ExternalOutput")

    def bc(ap, n):
        # append a stride-0 axis of size n (free-axis broadcast)
        return AP(ap.tensor, ap.offset, list(ap.ap) + [[0, n]])

    with tile.TileContext(nc) as tc:
        with ExitStack() as ctx:
            singles = ctx.enter_context(tc.tile_pool(name="singles", bufs=1))
            sc_pool = ctx.enter_context(tc.tile_pool(name="sc", bufs=3, space="PSUM"))
            ov_pool = ctx.enter_context(tc.tile_pool(name="ov", bufs=2, space="PSUM"))
            sm_pool = ctx.enter_context(tc.tile_pool(name="sm", bufs=3))

            qt_all = singles.tile([CHUNK, B, NREP], bf16, tag="qta")
            nc.scalar.dma_start(out=qt_all, in_=qt[:, :, :])
            ksc_all = singles.tile([CHUNK, B, CACHE // CHUNK], f32, tag="ksa")
            nc.scalar.dma_start(out=ksc_all, in_=ksc[:, :, :])
            vsc_all = singles.tile([CHUNK, B, CACHE // CHUNK], f32, tag="vsa")
            nc.scalar.dma_start(out=vsc_all, in_=vsc[:, :, :])
            out_num = singles.tile([CHUNK, B, NREP], f32, tag="onum")
            den_vec = singles.tile([CHUNK, B, NREP], f32, tag="oden")
            nc.vector.memset(den_vec, 0.0)

            # all K/V tiles resident; both DMA streams issued up front
            ktsups, vsups = [], []
            for b in range(B):
                nch = ncs[b]
                ngg = -(-nch // VG)
                ksrc = kt16[b, :, 0 : nch * CHUNK].rearrange(
                    "d (c s) -> d c s", s=CHUNK
                )
                ktsup = singles.tile(
                    [CHUNK, nch, CHUNK], bf16, tag=f"k{b}", name=f"ktsup{b}"
                )
                nc.sync.dma_start(out=ktsup, in_=ksrc)
                ktsups.append(ktsup)
                vsrc = v8[b, 0:ngg].rearrange("g p j d -> p g j d")
                vsup = singles.tile(
                    [CHUNK, ngg, VG, CHUNK], bf16, tag=f"v{b}", name=f"vsup{b}"
                )
                nc.gpsimd.dma_start(out=vsup, in_=vsrc)
                vsups.append(vsup)

            for b in range(B):
                    nch = ncs[b]
                    ktsup = ktsups[b]
                    vsup = vsups[b]
                    ngrp = -(-nch // G)
                    grps = [(k * G, min(G, nch - k * G)) for k in range(ngrp)]
                    ov = ov_pool.tile([CHUNK, NREP], f32)
                    pxs = [None] * ngrp

                    def front(k):
                        c0, gsz = grps[k]
                        scb = sc_pool.tile([CHUNK, G, NREP], f32)
                        for g in range(gsz):
                            nc.tensor.matmul(
                                scb[:, g, :],
                                lhsT=ktsup[:, c0 + g, :],
                                rhs=qt_all[:, b, :],
                                start=True,
                                stop=True,
                            )
                        sc_v = scb[:, 0:gsz, :]
                        nc.vector.tensor_tensor(
                            out=sc_v,
                            in0=sc_v,
                            in1=bc(ksc_all[:, b, c0 : c0 + gsz], NREP),
                            op=mult,
                        )
                        px = sm_pool.tile([CHUNK, G, NREP], bf16, tag="px")
                        px_v = px[:, 0:gsz, :]
                        nc.scalar.activation(
                            px_v, sc_v, mybir.ActivationFunctionType.Exp
                        )
                        tmp = sm_pool.tile([CHUNK, NREP], f32, tag="tmp")
                        nc.vector.tensor_reduce(
                            tmp,
                            px_v.rearrange("p g r -> p r g"),
                            axis=mybir.AxisListType.X,
                            op=add,
                        )
                        nc.vector.tensor_tensor(
                            out=den_vec[:, b, :],
                            in0=den_vec[:, b, :],
                            in1=tmp,
                            op=add,
                        )
                        nc.vector.tensor_tensor(
                            out=px_v,
                            in0=px_v,
                            in1=bc(vsc_all[:, b, c0 : c0 + gsz], NREP),
                            op=mult,
                        )
                        pxs[k] = px

                    def back(k):
                        c0, gsz = grps[k]
                        px = pxs[k]
                        for g in range(gsz):
                            c = c0 + g
                            nc.tensor.matmul(
                                ov,
                                lhsT=vsup[:, c // VG, c % VG, :],
                                rhs=px[:, g, :],
                                start=(c == 0),
                                stop=(c == nch - 1),
                            )
                        pxs[k] = None

                    front(0)
                    for k in range(1, ngrp):
                        front(k)
                        back(k - 1)
                    back(ngrp - 1)

                    nc.vector.tensor_copy(out_num[:, b, :], ov)

            nc.sync.dma_start(out=o_num[:, :, :], in_=out_num)
            nc.sync.dma_start(out=o_den[:, :, :], in_=den_vec)

    nc.compile()
    return nc


def kernel(
    xq,
    xk,
    xv,
    freqs_cos,
    freqs_sin,
    k_scaler,
    v_scaler,
    cache_k,
    cache_v,
    input_pos,
):
    global LAST_RESULTS
    import ml_dtypes
    from concourse.bass_utils import run_bass_kernel_spmd

    bf16 = ml_dtypes.bfloat16
    xq = np.asarray(xq, np.float32)
    xk = np.asarray(xk, np.float32)
    xv = np.asarray(xv, np.float32)
    freqs_cos = np.asarray(freqs_cos, np.float32)
    freqs_sin = np.asarray(freqs_sin, np.float32)
    k_scaler = np.asarray(k_scaler, np.float32)
    v_scaler = np.asarray(v_scaler, np.float32)
    cache_k = np.asarray(cache_k)
    cache_v = np.asarray(cache_v)
    input_pos = np.asarray(input_pos)
    pos = input_pos.astype(np.int64)

    # --- tiny host prep: RoPE + int8 quantization of the single new token ---
    q = _rope(xq, freqs_cos, freqs_sin)[:, 0]  # [B, H, D]
    k = _rope(xk, freqs_cos, freqs_sin)[:, 0]  # [B, KVH, D]
    v_new = xv[:, 0]  # [B, KVH, D]
    k_s = (np.max(np.abs(k), axis=-1, keepdims=True) / np.float32(127.0)).astype(
        np.float32
    ) + np.float32(1e-8)
    v_s = (np.max(np.abs(v_new), axis=-1, keepdims=True) / np.float32(127.0)).astype(
        np.float32
    ) + np.float32(1e-8)
    k_q = np.clip(np.round(k / k_s), -127, 127).astype(np.int8)
    v_q = np.clip(np.round(v_new / v_s), -127, 127).astype(np.int8)

    # device batch order: ascending chunk count (shrinks DMA-group padding)
    ncs_raw = pos // CHUNK + 1
    order = np.argsort(ncs_raw, kind="stable")
    ncs = tuple(int(ncs_raw[b]) for b in order)

    if ncs not in _BUILD_CACHE:
        _BUILD_CACHE[ncs] = _build_program(ncs)
    nc = _BUILD_CACHE[ncs]

    bidx = np.arange(B)
    inv_sqrt_d = np.float32(1.0 / math.sqrt(D))
    s_idx = np.arange(CACHE, dtype=np.int64)
    masked = s_idx[None, :] > pos[:, None]  # [B, CACHE] True -> excluded
    masked_dev = masked[order]
    pos_dev = pos[order]
    # masked positions inside processed chunks contribute exp(0)=1 each
    n_masked = (np.asarray(ncs, np.int64) * CHUNK - (pos_dev + 1)).astype(
        np.float32
    )  # [B] device order

    def chunk_layout(a):  # [B, CACHE] -> [128, B, 32] with s = c*128 + p
        return np.ascontiguousarray(
            a.reshape(B, CACHE // CHUNK, CHUNK).transpose(2, 0, 1)
        )

    in_maps = []
    for m in range(NCORES):
        ck_m = cache_k[:, m].astype(np.int8)  # [B, CACHE, D]
        cv_m = cache_v[:, m].astype(np.int8)
        ck_m[bidx, pos, :] = k_q[:, m]
        cv_m[bidx, pos, :] = v_q[:, m]
        ck_m = ck_m[order].astype(bf16)
        cv8 = cv_m[order]

        kt16 = np.ascontiguousarray(ck_m.transpose(0, 2, 1))  # [B, D, CACHE]
        v8 = np.ascontiguousarray(
            cv8.reshape(B, CACHE // (VG * CHUNK), VG, CHUNK, D).transpose(
                0, 1, 3, 2, 4
            )
        )  # [B, 8, 128, VG, D] int8

        ks_m = k_scaler[:, m].copy()  # [B, CACHE]
        vs_m = v_scaler[:, m].copy()
        ks_m[bidx, pos] = k_s[:, m, 0]
        vs_m[bidx, pos] = v_s[:, m, 0]
        ks_m = ks_m[order]
        vs_m = vs_m[order]

        ks_m *= inv_sqrt_d
        ks_m[masked_dev] = np.float32(0.0)
        vs_m[masked_dev] = np.float32(0.0)

        qt_m = np.ascontiguousarray(
            q[order][:, m * NREP : (m + 1) * NREP, :].transpose(2, 0, 1)
        ).astype(bf16)  # [D, B, NREP]

        in_maps.append(
            dict(
                kt16=kt16,
                v8=v8,
                qt=qt_m,
                ksc=chunk_layout(ks_m),
                vsc=chunk_layout(vs_m),
            )
        )

    res = run_bass_kernel_spmd(nc, in_maps, core_ids=list(range(NCORES)))
    LAST_RESULTS = res

    inv_order = np.empty(B, np.int64)
    inv_order[order] = np.arange(B)
    out = np.zeros((B, H, 1, D), np.float32)
    for m in range(NCORES):
        num = np.asarray(res.results[m]["o_num"], np.float32)  # [D, B, NREP]
        dvec = np.asarray(res.results[m]["o_den"], np.float32)  # [128, B, NREP]
        den = dvec.sum(axis=0) - n_masked[:, None]  # [B, NREP]
        o = (num / den[None, :, :]).transpose(1, 2, 0)  # [B, NREP, D]
        out[:, m * NREP : (m + 1) * NREP, 0, :] = o[inv_order]
    return out


# revision 14
# speedup vs baseline: 1.0993x; 1.0993x over previous
"""GQA int8-KV-cache decode attention on 8 NeuronCores (Bass/Tile), v3.

Sharding: kv-head axis (8 kv heads -> 1 per core), tensor parallel.
Host does tiny prep (RoPE + int8 quant of the new token, cache patch,
layout shuffles); device does the masked attention sweep.

Layout/algorithm choices:
  * K/V caches shipped to HBM as bf16 (cast on host). The DMA engines
    charge the max(src,dst) side, so an int8->bf16 cast DMA costs the
    same as a raw bf16 copy -- but raw copies ride the hardware DGE
    (no ~1us/DMA SWDGE desc-gen on GpSimd, no queue DRAIN stalls), so
    exact per-batch loads with no group padding become free.
  * K stored pre-transposed [B, D, CACHE] so each chunk arrives as
    KT [d, s] and feeds matmul directly -- no PE transpose.
  * V stored bf16 position-interleaved [B, 16, 128, 2, D] so SBUF
    partition p holds positions {g*256 + j*128 + p}: per-partition
    contiguous runs are 512B (full DMA bandwidth) while the SBUF tile
    keeps partition = position-within-chunk.
  * All K/V DMAs ride the sync HWDGE queue, strictly interleaved
    K0,V0,K1,V1,... so tiles complete in exactly the order compute
    consumes them. Every batch gets its own resident SBUF tile (the
    full working set fits), so no buffer-reuse WAR can delay an issue;
    issues just block on DMA-engine backpressure, which is harmless on
    a queue with nothing else on it. The scalar queue stays clean for
    the exp ACTIVATEs (a DMA queued behind ACTIVATEs gets throttled to
    compute pace; a scalar queue full of blocking DMA issues starves
    the exps -- both measured).
  * Scores for G=8 chunks accumulate into one PSUM tile; the k_scaler
    multiply is a single DVE tensor_tensor against a stride-0-broadcast
    [128, G, 1->4] AP, then ONE batched ACT exp per G chunks (the ACT
    per-instruction PSUM-access overhead was the v2 bottleneck).
  * Mask via scalers only: masked positions have k_scaler = 0 (so
    exp(0) = 1 exactly) and v_scaler = 0 (no numerator contribution);
    the denominator over-count is the masked-position count, which the
    host subtracts exactly. No mask tensor, no -inf bias.
  * Denominator = sum_s exp accumulated as a per-partition vector on
    DVE (reduce over chunks + add); host does the final 128-partition
    sum. No per-chunk PE matmul for the denominator.
  * Numerator ov[d, r] += V_chunk.T @ pexp accumulates in PSUM across
    chunks; numerator and denominator ship out unnormalized, host
    divides.
  * scores->exp->PV chain software-pipelined at group granularity so
    the PE never head-of-line blocks on ACT/DVE.
"""

import os

os.environ.setdefault("JAX_PLATFORMS", "cpu")

import math
import numpy as np

B, H, KVH, D, CACHE = 16, 32, 8, 128, 4096
NREP = H // KVH
NCORES = 8
CHUNK = 128
VG = 2  # V chunks interleaved per 512B-run group (bf16)
NBG = 4  # batches per DMA group
G = int(os.environ.get("KERNEL_G", "8"))  # chunks per ACT/exp batch

_BUILD_CACHE = {}
LAST_RESULTS = None


def _rope(x, cos, sin):
    # x: [B, 1, Hx, D]; cos/sin: [B, 1, D//2]
    c = cos[:, :, None, :]
    s = sin[:, :, None, :]
    xe, xo = x[..., ::2], x[..., 1::2]
    re = xe * c - xo * s
    im = xe * s + xo * c
    return np.stack([re, im], axis=-1).reshape(x.shape).astype(np.float32)


def _build_program(ncs):
    """ncs: per-device-batch chunk counts, sorted ascending (same for
    every core since sharding is by kv head)."""
    from contextlib import ExitStack

    import concourse.bacc as bacc
    import concourse.tile as tile
    from concourse import mybir
    from concourse.bass import AP

    nc = bacc.Bacc()
    f32 = mybir.dt.float32
    bf16 = mybir.dt.bfloat16
    mult = mybir.AluOpType.mult
    add = mybir.AluOpType.add

    kt16 = nc.dram_tensor("kt16", [B, D, CACHE], bf16, kind="ExternalInput")
    v16 = nc.dram_tensor(
        "v16", [B, CACHE // (VG * CHUNK), CHUNK, VG, D], bf16, kind="ExternalInput"
    )
    qt = nc.dram_tensor("qt", [CHUNK, B, NREP], bf16, kind="ExternalInput")
    ksc = nc.dram_tensor("ksc", [CHUNK, B, CACHE // CHUNK], f32, kind="ExternalInput")
    vsc = nc.dram_tensor("vsc", [CHUNK, B, CACHE // CHUNK], f32, kind="ExternalInput")
    o_num = nc.dram_tensor("o_num", [CHUNK, B, NREP], f32, kind="ExternalOutput")
    o_den = nc.dram_tensor("o_den", [CHUNK, B, NREP], f32, kind="ExternalOutput")

    def bc(ap, n):
        # append a stride-0 axis of size n (free-axis broadcast)
        return AP(ap.tensor, ap.offset, list(ap.ap) + [[0, n]])

    with tile.TileContext(nc) as tc:
        with ExitStack() as ctx:
            singles = ctx.enter_context(tc.tile_pool(name="singles", bufs=1))
            sc_pool = ctx.enter_context(tc.tile_pool(name="sc", bufs=3, space="PSUM"))
            ov_pool = ctx.enter_context(tc.tile_pool(name="ov", bufs=2, space="PSUM"))
            sm_pool = ctx.enter_context(tc.tile_pool(name="sm", bufs=3))

            qt_all = singles.tile([CHUNK, B, NREP], bf16, tag="qta")
            nc.scalar.dma_start(out=qt_all, in_=qt[:, :, :])
            ksc_all = singles.tile([CHUNK, B, CACHE // CHUNK], f32, tag="ksa")
            nc.scalar.dma_start(out=ksc_all, in_=ksc[:, :, :])
            vsc_all = singles.tile([CHUNK, B, CACHE // CHUNK], f32, tag="vsa")
            nc.scalar.dma_start(out=vsc_all, in_=vsc[:, :, :])
            out_num = singles.tile([CHUNK, B, NREP], f32, tag="onum")
            den_vec = singles.tile([CHUNK, B, NREP], f32, tag="oden")
            nc.vector.memset(den_vec, 0.0)

            # all K/V tiles resident; both DMA streams issued up front
            ktsups, vsups = [], []
            for b in range(B):
                nch = ncs[b]
                ngg = -(-nch // VG)
                ksrc = kt16[b, :, 0 : nch * CHUNK].rearrange(
                    "d (c s) -> d c s", s=CHUNK
                )
                ktsup = singles.tile(
                    [CHUNK, nch, CHUNK], bf16, tag=f"k{b}", name=f"ktsup{b}"
                )
                nc.sync.dma_start(out=ktsup, in_=ksrc)
                ktsups.append(ktsup)
                vsrc = v16[b, 0:ngg].rearrange("g p j d -> p g j d")
                vsup = singles.tile(
                    [CHUNK, ngg, VG, CHUNK], bf16, tag=f"v{b}", name=f"vsup{b}"
                )
                nc.sync.dma_start(out=vsup, in_=vsrc)
                vsups.append(vsup)

            for b in range(B):
                    nch = ncs[b]
                    ktsup = ktsups[b]
                    vsup = vsups[b]
                    ngrp = -(-nch // G)
                    grps = [(k * G, min(G, nch - k * G)) for k in range(ngrp)]
                    ov = ov_pool.tile([CHUNK, NREP], f32)
                    pxs = [None] * ngrp

                    def front(k):
                        c0, gsz = grps[k]
                        scb = sc_pool.tile([CHUNK, G, NREP], f32)
                        for g in range(gsz):
                            nc.tensor.matmul(
                                scb[:, g, :],
                                lhsT=ktsup[:, c0 + g, :],
                                rhs=qt_all[:, b, :],
                                start=True,
                                stop=True,
                            )
                        sc_v = scb[:, 0:gsz, :]
                        nc.vector.tensor_tensor(
                            out=sc_v,
                            in0=sc_v,
                            in1=bc(ksc_all[:, b, c0 : c0 + gsz], NREP),
                            op=mult,
                        )
                        px = sm_pool.tile([CHUNK, G, NREP], bf16, tag="px")
                        px_v = px[:, 0:gsz, :]
                        nc.scalar.activation(
                            px_v, sc_v, mybir.ActivationFunctionType.Exp
                        )
                        tmp = sm_pool.tile([CHUNK, NREP], f32, tag="tmp")
                        nc.vector.tensor_reduce(
                            tmp,
                            px_v.rearrange("p g r -> p r g"),
                            axis=mybir.AxisListType.X,
                            op=add,
                        )
                        nc.vector.tensor_tensor(
                            out=den_vec[:, b, :],
                            in0=den_vec[:, b, :],
                            in1=tmp,
                            op=add,
                        )
                        nc.vector.tensor_tensor(
                            out=px_v,
                            in0=px_v,
                            in1=bc(vsc_all[:, b, c0 : c0 + gsz], NREP),
                            op=mult,
                        )
                        pxs[k] = px

                    def back(k):
                        c0, gsz = grps[k]
                        px = pxs[k]
                        for g in range(gsz):
                            c = c0 + g
                            nc.tensor.matmul(
                                ov,
                                lhsT=vsup[:, c // VG, c % VG, :],
                                rhs=px[:, g, :],
                                start=(c == 0),
                                stop=(c == nch - 1),
                            )
                        pxs[k] = None

                    front(0)
                    for k in range(1, ngrp):
                        front(k)
                        back(k - 1)
                    back(ngrp - 1)

                    nc.vector.tensor_copy(out_num[:, b, :], ov)

            nc.sync.dma_start(out=o_num[:, :, :], in_=out_num)
            nc.sync.dma_start(out=o_den[:, :, :], in_=den_vec)

    nc.compile()
    return nc


def kernel(
    xq,
    xk,
    xv,
    freqs_cos,
    freqs_sin,
    k_scaler,
    v_scaler,
    cache_k,
    cache_v,
    input_pos,
):
    global LAST_RESULTS
    import ml_dtypes
    from concourse.bass_utils import run_bass_kernel_spmd

    bf16 = ml_dtypes.bfloat16
    xq = np.asarray(xq, np.float32)
    xk = np.asarray(xk, np.float32)
    xv = np.asarray(xv, np.float32)
    freqs_cos = np.asarray(freqs_cos, np.float32)
    freqs_sin = np.asarray(freqs_sin, np.float32)
    k_scaler = np.asarray(k_scaler, np.float32)
    v_scaler = np.asarray(v_scaler, np.float32)
    cache_k = np.asarray(cache_k)
    cache_v = np.asarray(cache_v)
    input_pos = np.asarray(input_pos)
    pos = input_pos.astype(np.int64)

    # --- tiny host prep: RoPE + int8 quantization of the single new token ---
    q = _rope(xq, freqs_cos, freqs_sin)[:, 0]  # [B, H, D]
    k = _rope(xk, freqs_cos, freqs_sin)[:, 0]  # [B, KVH, D]
    v_new = xv[:, 0]  # [B, KVH, D]
    k_s = (np.max(np.abs(k), axis=-1, keepdims=True) / np.float32(127.0)).astype(
        np.float32
    ) + np.float32(1e-8)
    v_s = (np.max(np.abs(v_new), axis=-1, keepdims=True) / np.float32(127.0)).astype(
        np.float32
    ) + np.float32(1e-8)
    k_q = np.clip(np.round(k / k_s), -127, 127).astype(np.int8)
    v_q = np.clip(np.round(v_new / v_s), -127, 127).astype(np.int8)

    # device batch order: ascending chunk count (shrinks DMA-group padding)
    ncs_raw = pos // CHUNK + 1
    order = np.argsort(ncs_raw, kind="stable")
    ncs = tuple(int(ncs_raw[b]) for b in order)

    if ncs not in _BUILD_CACHE:
        _BUILD_CACHE[ncs] = _build_program(ncs)
    nc = _BUILD_CACHE[ncs]

    bidx = np.arange(B)
    inv_sqrt_d = np.float32(1.0 / math.sqrt(D))
    s_idx = np.arange(CACHE, dtype=np.int64)
    masked = s_idx[None, :] > pos[:, None]  # [B, CACHE] True -> excluded
    masked_dev = masked[order]
    pos_dev = pos[order]
    # masked positions inside processed chunks contribute exp(0)=1 each
    n_masked = (np.asarray(ncs, np.int64) * CHUNK - (pos_dev + 1)).astype(
        np.float32
    )  # [B] device order

    def chunk_layout(a):  # [B, CACHE] -> [128, B, 32] with s = c*128 + p
        return np.ascontiguousarray(
            a.reshape(B, CACHE // CHUNK, CHUNK).transpose(2, 0, 1)
        )

    in_maps = []
    for m in range(NCORES):
        ck_m = cache_k[:, m].astype(np.int8)  # [B, CACHE, D]
        cv_m = cache_v[:, m].astype(np.int8)
        ck_m[bidx, pos, :] = k_q[:, m]
        cv_m[bidx, pos, :] = v_q[:, m]
        ck_m = ck_m[order].astype(bf16)
        cv16 = cv_m[order].astype(bf16)

        kt16 = np.ascontiguousarray(ck_m.transpose(0, 2, 1))  # [B, D, CACHE]
        v16 = np.ascontiguousarray(
            cv16.reshape(B, CACHE // (VG * CHUNK), VG, CHUNK, D).transpose(
                0, 1, 3, 2, 4
            )
        )  # [B, 16, 128, 2, D] bf16

        ks_m = k_scaler[:, m].copy()  # [B, CACHE]
        vs_m = v_scaler[:, m].copy()
        ks_m[bidx, pos] = k_s[:, m, 0]
        vs_m[bidx, pos] = v_s[:, m, 0]
        ks_m = ks_m[order]
        vs_m = vs_m[order]

        ks_m *= inv_sqrt_d
        ks_m[masked_dev] = np.float32(0.0)
        vs_m[masked_dev] = np.float32(0.0)

        qt_m = np.ascontiguousarray(
            q[order][:, m * NREP : (m + 1) * NREP, :].transpose(2, 0, 1)
        ).astype(bf16)  # [D, B, NREP]

        in_maps.append(
            dict(
                kt16=kt16,
                v16=v16,
                qt=qt_m,
                ksc=chunk_layout(ks_m),
                vsc=chunk_layout(vs_m),
            )
        )

    res = run_bass_kernel_spmd(nc, in_maps, core_ids=list(range(NCORES)))
    LAST_RESULTS = res

    inv_order = np.empty(B, np.int64)
    inv_order[order] = np.arange(B)
    out = np.zeros((B, H, 1, D), np.float32)
    for m in range(NCORES):
        num = np.asarray(res.results[m]["o_num"], np.float32)  # [D, B, NREP]
        dvec = np.asarray(res.results[m]["o_den"], np.float32)  # [128, B, NREP]
        den = dvec.sum(axis=0) - n_masked[:, None]  # [B, NREP]
        o = (num / den[None, :, :]).transpose(1, 2, 0)  # [B, NREP, D]
        out[:, m * NREP : (m + 1) * NREP, 0, :] = o[inv_order]
    return out
